# revision 25
# baseline (speedup 1.0000x reference)
"""Trainium2 Bass kernel for nn_Demolition_splitweight_Conv2d.

Computation (per batch element b, one NeuronCore each):
    out[o, p] = (1/(127*Q)) * sum_k wvec[k] * sum_c round(Q*(conv3x3(x[c]; w[k,c,o]) + b[k,c,o]))
with Q = 12.5, wvec = [-128, 1, 2, 4, 8, 16, 32, 64].

Scheme: fp16 single-term products + fp32 magic-number rounding inside the
TensorEngine accumulation. fp16's 11-bit significand makes Dekker splitting
unnecessary (measured rel err ~9e-3 vs the 2e-2 gate), so each input channel
needs only 12 contraction rows:
    [9 tap rows, bias row (rhs=ones), +M row, -M row]   (M = 1.5*2^23 = 3072*4096)

HW probe result (probe_seg32.py): the PE accumulates a matmul's contraction
rows sequentially WITHIN 32-row segments, then combines segment totals
atomically (and likewise chained matmuls combine atomically through PSUM).
A magic-rounding unit therefore must sit entirely inside one 32-row segment:
2 channels per segment (24 rows + 8 zero pad), 8 channels per pass, 4 passes
x 2 output halves (m in {0,1}: out cols (k,ol) = 8 bits x 16 channels) per
pixel block. The 4th segment needs no pad: contraction K = 120.

Layout: data-parallel over batch (8 cores). The host pre-shifts each
channel's zero-padded image by each tap offset and interleaves bias-ones /
magic-4096 / pad-zero rows, sliced per pixel block: one SBUF tile per block
[120, 4 passes x window], uploaded as one contiguous full-rate DMA each.
Per-block tiles keep the tile-framework's whole-tile dependency exact, which
lets stage-1 matmuls use a 3D rhs AP that skips the 2 pad columns per image
row (dense moving dim = rows x 64 px).

Block sizes ramp 3,5,6 image rows then 8s (sim-swept): the upload of a
block (176 ns/row) always lands before the PE needs it (267 ns/row), so
after the boot DMA (weights + block 0) the PE almost never gaps (gaps can
reset the PE p-state ramp). A tiny final block (2 rows) shortens the
copy->DMA tail, and the output is written as fp16 (values are O(1); adds
~2e-4 rel err) to halve output-DMA bytes. Stage 2 (bit recombination via
the wvec*SCALE matrix; PSUM -> fp16 A copies on vector/scalar) interleaves
in PE program order after the NEXT block's stage 1 so the PE never waits
on copy latency. Dep-free warmup matmuls ramp the PE p-state during the
boot DMA. Measured: 25852 ns (TimelineSim), rel err 9.3e-3 on HW
(baseline bf16 3-term kernel: 44011 ns, 1.1e-3).
"""

import numpy as np

import concourse.bass as bass
import concourse.mybir as mybir
from concourse.ap import AP
from concourse.tile import TileContext
from concourse.bass_utils import run_bass_kernel_spmd

# problem dims (hardcoded per the task contract)
B, C, OUT, H, W = 8, 32, 32, 64, 64
KBITS = 8
Q = 12.5
WVEC = np.array([-128, 1, 2, 4, 8, 16, 32, 64], np.float32)
SCALE = float(1.0 / (127.0 * Q))

PW = 66            # padded width  (1 + 64 + 1)
PH = 66            # padded height
PSZ = PH * PW      # 4356
NG = 4             # stage-1 passes: 4 x (4 segments x 2 channels)
KC = 120           # contraction rows per pass (last segment unpadded)
RPC = 12           # contraction rows per channel

BLK_ROWS = [3, 5, 6, 8, 8, 8, 8, 8, 8, 2]         # image rows per block
NPB = len(BLK_ROWS)
BLK_R0 = np.cumsum([0] + BLK_ROWS).tolist()        # first image row
MAXR = max(BLK_ROWS)
NDEN = MAXR * W    # max dense moving dim: 512 = one PSUM bank exactly
PADW = 64 * PW + 134  # host pad buffer width

NWARM = 30          # PE p-state warmup matmuls (128 cols each)
WCW = 2 * NG * 128 + 64  # weight cols: 8 stage-1 lhsT blocks + gmat
BOOTW = WCW + NG * BLK_ROWS[0] * PW  # boot: weights + pixel block 0
XREPW = NG * (H - BLK_ROWS[0]) * PW  # xrep dram: blocks 1..NPB-1

F16 = mybir.dt.float16
F32 = mybir.dt.float32

_cache = {}


def _f16(a):
    return np.asarray(a, np.float32).astype(np.float16)


def _row_of(c, j):
    """Contraction row (pass g, partition w) for channel c, intra row j."""
    g, r = divmod(c, 8)
    s, t = divmod(r, 2)
    return g, s * 32 + t * RPC + j


def _prep_weights(weight, bias):
    """Weight cols [128, WCW] fp16: 8 stage-1 lhsT blocks (g,m) + gmat."""
    qw = _f16(Q * weight.astype(np.float32)).reshape(KBITS, C, 2, 16, 9)
    qb = _f16(Q * bias.astype(np.float32)).reshape(KBITS, C, 2, 16)

    Wt = np.zeros((NG, KC, 2, 128), np.float16)
    for c in range(C):
        for j in range(9):
            g, w = _row_of(c, j)
            # qw[k,c,m,ol] -> [m, (k*16+ol)]
            Wt[g, w] = qw[:, c, :, :, j].transpose(1, 0, 2).reshape(2, 128)
        g, w = _row_of(c, 9)
        Wt[g, w] = qb[:, c].transpose(1, 0, 2).reshape(2, 128)
        g, w = _row_of(c, 10)
        Wt[g, w] = np.float16(3072.0)
        g, w = _row_of(c, 11)
        Wt[g, w] = np.float16(-3072.0)

    wc = np.zeros((128, WCW), np.float16)
    for g in range(NG):
        for m in range(2):
            wc[:KC, (g * 2 + m) * 128:(g * 2 + m + 1) * 128] = Wt[g, :, m, :]

    j = np.arange(128)
    k_of, ol_of = j // 16, j % 16
    gmat = np.zeros((128, 64), np.float32)
    gmat[j, ol_of] = WVEC[k_of] * SCALE           # A0 -> out rows 0..15
    gmat[j, 32 + 16 + ol_of] = WVEC[k_of] * SCALE  # A1 -> out rows 16..31
    wc[:, 2 * NG * 128:] = _f16(gmat)
    return wc


def _build_xrep(x):
    """Host REP: [B, KC, sum_blocks NG*window] fp16, block-sliced."""
    xh = _f16(x)
    xpad = np.zeros((B, C, PADW), np.float16)
    xpad[:, :, :PSZ].reshape(B, C, PH, PW)[:, :, 1:H + 1, 1:W + 1] = xh

    rep = np.zeros((B, NG, KC, H * PW), np.float16)
    for j in range(9):
        off = (j // 3) * PW + (j % 3)
        for c in range(C):
            g, w = _row_of(c, j)
            rep[:, g, w, :] = xpad[:, c, off:off + H * PW]
    for c in range(C):
        g, w = _row_of(c, 9)
        rep[:, g, w, :] = np.float16(1.0)
        for j in (10, 11):
            g, w = _row_of(c, j)
            rep[:, g, w, :] = np.float16(4096.0)
    # block-sliced, per block: [KC, NG * nr * PW]
    parts = []
    for pb in range(NPB):
        r0, nr = BLK_R0[pb], BLK_ROWS[pb]
        sl = rep[:, :, :, r0 * PW:(r0 + nr) * PW]           # [B, NG, KC, w]
        parts.append(sl.transpose(0, 2, 1, 3).reshape(B, KC, NG * nr * PW))
    return parts


def _split_multiwaits(nc):
    """This container's walrus allows one sync-wait per instruction; move
    extras onto preceding same-engine NoOps."""
    for bb in nc.main_func.blocks:
        insts = bb.instructions
        i = 0
        while i < len(insts):
            ins = insts[i]
            si = getattr(ins, "sync_info", None)
            if si is not None and si.on_wait is not None and len(si.on_wait) > 1:
                waits = list(si.on_wait)
                nops = []
                for j, w in enumerate(waits[:-1]):
                    nop = mybir.InstNoOp(name=f"{ins.name}-wsplit{j}", ins=[], outs=[])
                    nop.engine = ins.engine
                    nop.sync_info = mybir.SyncInfo(on_wait=[w], on_update=[])
                    nops.append(nop)
                si.on_wait = [waits[-1]]
                ins.sync_info = si
                for j, nop in enumerate(nops):
                    insts.insert(i + j, nop)
                i += len(nops)
            i += 1


def _build_nc():
    nc = bass.Bass()
    boot_d = nc.dram_tensor("boot", [128, BOOTW], F16, kind="ExternalInput")
    xrep_d = nc.dram_tensor("xrep", [KC, XREPW], F16, kind="ExternalInput")
    out_d = nc.dram_tensor("out", [OUT, H * W], F16, kind="ExternalOutput")

    with TileContext(nc) as tc:
        with (
            tc.tile_pool(name="const", bufs=1) as cpool,
            tc.tile_pool(name="blk", bufs=1) as bpool,
            tc.tile_pool(name="work", bufs=4) as wpool,
            tc.tile_pool(name="outp", bufs=2) as opool,
            tc.tile_pool(name="psP", bufs=4, space="PSUM") as psP,
            tc.tile_pool(name="psR", bufs=3, space="PSUM") as psR,
        ):
            # warmup: PE p-state ramp on a zero tile (memset on the
            # otherwise-idle Pool engine for the shortest dep latency)
            warm = cpool.tile([128, 128], F16, tag="warm")
            nc.gpsimd.memset(warm[:, :], 0.0)
            warm_ps = psP.tile([128, NDEN], F32, tag="P", name="warmps")
            for wi in range(NWARM):
                nc.tensor.matmul(warm_ps[:, :128], warm[:, :], warm[:, :],
                                 start=True, stop=True)

            boot = cpool.tile([128, BOOTW], F16, tag="boot")
            nc.sync.dma_start(out=boot[:, :], in_=boot_d[:, :])
            wconst = boot[:, 0:WCW]

            blk = [None] * NPB
            off_x = 0
            for pb in range(1, NPB):
                w = NG * BLK_ROWS[pb] * PW
                blk[pb] = bpool.tile([KC, w], F16, tag=f"blk{pb}",
                                     name=f"blk{pb}")
                src = AP(tensor=xrep_d, offset=off_x, ap=[[XREPW, KC], [1, w]])
                dst = AP(tensor=blk[pb].tensor, offset=blk[pb].offset,
                         ap=[[w, KC], [1, w]])
                nc.sync.dma_start(out=dst, in_=src)
                off_x += w

            def stage1(pb):
                nr = BLK_ROWS[pb]
                n = nr * W
                gw = nr * PW
                P = [psP.tile([128, NDEN], F32, tag="P", name=f"P{pb}_{m}")
                     for m in range(2)]
                for g in range(NG):
                    if pb == 0:
                        t, off, pitch = boot, boot.offset + WCW + g * gw, BOOTW
                    else:
                        t = blk[pb]
                        off, pitch = t.offset + g * gw, NG * gw
                    rhs = AP(tensor=t.tensor, offset=off,
                             ap=[[pitch, KC], [PW, nr], [1, W]])
                    for m in range(2):
                        lt = boot[0:KC, (g * 2 + m) * 128:(g * 2 + m + 1) * 128]
                        nc.tensor.matmul(P[m][:, :n], lt, rhs,
                                         start=(g == 0), stop=(g == NG - 1))
                A = [wpool.tile([128, NDEN], F16, tag="A", name=f"A{pb}_{m}")
                     for m in range(2)]
                # last (tiny) block: both copies on DVE — the scalar engine
                # is still busy with the previous block's copy
                nc.vector.tensor_copy(A[0][:, :n], P[0][:, :n])
                if pb == NPB - 1:
                    nc.vector.tensor_copy(A[1][:, :n], P[1][:, :n])
                else:
                    nc.scalar.copy(A[1][:, :n], P[1][:, :n])
                return A

            osb = None
            osb_off = 0
            osb_pb0 = 0
            # output-DMA pairing: if NPB is odd, block 0 goes solo (its DMA
            # fires early and costs nothing); later blocks pair up so the
            # tail ends with exactly one DMA
            pair_start = [True] + [(pb % 2) == (NPB % 2) for pb in range(1, NPB)]

            def stage2(pb, A):
                nonlocal osb, osb_off, osb_pb0
                nr = BLK_ROWS[pb]
                n = nr * W
                R = psR.tile([32, NDEN], F32, tag="R", name=f"R{pb}")
                gm0 = wconst[:, 2 * NG * 128:2 * NG * 128 + 32]
                gm1 = wconst[:, 2 * NG * 128 + 32:2 * NG * 128 + 64]
                nc.tensor.matmul(R[:, :n], gm0, A[0][:, :n],
                                 start=True, stop=False)
                nc.tensor.matmul(R[:, :n], gm1, A[1][:, :n],
                                 start=False, stop=True)
                if pair_start[pb]:
                    osb = opool.tile([32, 2 * NDEN], F16, tag="osb",
                                     name=f"osb{pb}")
                    osb_off = 0
                    osb_pb0 = pb
                out_ap = AP(tensor=osb.tensor, offset=osb.offset + osb_off,
                            ap=[[2 * NDEN, 32], [1, n]])
                # alternate osb copies between engines; swap for the final
                # pair so the big half lands on the faster scalar engine
                on_dve = (pb % 2 == 0) if pb < NPB - 2 else (pb % 2 == 1)
                if on_dve:
                    nc.vector.tensor_copy(out_ap, R[:, :n])
                else:
                    nc.scalar.copy(out_ap, R[:, :n])
                osb_off += n
                if pb == NPB - 1 or pair_start[pb + 1]:
                    pb0 = osb_pb0
                    nrows = sum(BLK_ROWS[pb0:pb + 1])
                    dst = AP(tensor=out_d, offset=BLK_R0[pb0] * W,
                             ap=[[H * W, OUT], [1, nrows * W]])
                    src = AP(tensor=osb.tensor, offset=osb.offset,
                             ap=[[2 * NDEN, 32], [1, nrows * W]])
                    nc.sync.dma_start(out=dst, in_=src)

            prevA = None
            for pb in range(NPB):
                A = stage1(pb)
                if prevA is not None:
                    stage2(pb - 1, prevA)
                prevA = A
            stage2(NPB - 1, prevA)

    _split_multiwaits(nc)
    return nc


def kernel(x, weight, bias):
    x = np.asarray(x, np.float32)
    weight = np.asarray(weight, np.float32)
    bias = np.asarray(bias, np.float32)

    parts = _build_xrep(x)
    wc = _prep_weights(weight, bias)

    if "nc" not in _cache:
        _cache["nc"] = _build_nc()
    nc = _cache["nc"]

    in_maps = []
    for b in range(B):
        boot = np.zeros((128, BOOTW), np.float16)
        boot[:, :WCW] = wc
        boot[:KC, WCW:] = parts[0][b]
        xr = np.concatenate([p[b] for p in parts[1:]], axis=1)
        in_maps.append({"boot": boot, "xrep": xr})
    res = run_bass_kernel_spmd(nc, in_maps, core_ids=list(range(B)))
    out = np.stack([r["out"] for r in res.results])
    return out.reshape(B, OUT, H, W).astype(np.float32)


# revision 26
# speedup vs baseline: 1.0058x; 1.0058x over previous
"""Trainium2 Bass kernel for nn_Demolition_splitweight_Conv2d.

Computation (per batch element b, one NeuronCore each):
    out[o, p] = (1/(127*Q)) * sum_k wvec[k] * sum_c round(Q*(conv3x3(x[c]; w[k,c,o]) + b[k,c,o]))
with Q = 12.5, wvec = [-128, 1, 2, 4, 8, 16, 32, 64].

Scheme: fp16 single-term products + fp32 magic-number rounding inside the
TensorEngine accumulation. fp16's 11-bit significand makes Dekker splitting
unnecessary (measured rel err ~9e-3 vs the 2e-2 gate), so each input channel
needs only 12 contraction rows:
    [9 tap rows, bias row (rhs=ones), +M row, -M row]   (M = 1.5*2^23 = 3072*4096)

HW probe result (probe_seg32.py): the PE accumulates a matmul's contraction
rows sequentially WITHIN 32-row segments, then combines segment totals
atomically (and likewise chained matmuls combine atomically through PSUM).
A magic-rounding unit therefore must sit entirely inside one 32-row segment:
2 channels per segment (24 rows + 8 zero pad), 8 channels per pass, 4 passes
x 2 output halves (m in {0,1}: out cols (k,ol) = 8 bits x 16 channels) per
pixel block. The 4th segment needs no pad: contraction K = 120.

Layout: data-parallel over batch (8 cores). The host pre-shifts each
channel's zero-padded image by each tap offset and interleaves bias-ones /
magic-4096 / pad-zero rows, sliced per pixel block: one SBUF tile per block
[120, 4 passes x window], uploaded as one contiguous full-rate DMA each.
Per-block tiles keep the tile-framework's whole-tile dependency exact, which
lets stage-1 matmuls use a 3D rhs AP that skips the 2 pad columns per image
row (dense moving dim = rows x 64 px).

Block sizes ramp 3,5,6 image rows then 8s (sim-swept): the upload of a
block (176 ns/row) always lands before the PE needs it (267 ns/row), so
after the boot DMA (weights + block 0) the PE almost never gaps (gaps can
reset the PE p-state ramp). A tiny final block (2 rows) shortens the
copy->DMA tail, and the output is written as fp16 (values are O(1); adds
~2e-4 rel err) to halve output-DMA bytes. Stage 2 (bit recombination via
the wvec*SCALE matrix; PSUM -> fp16 A copies on vector/scalar) interleaves
in PE program order after the NEXT block's stage 1 so the PE never waits
on copy latency. Dep-free warmup matmuls ramp the PE p-state during the
boot DMA. Measured: 25852 ns (TimelineSim), rel err 9.3e-3 on HW
(baseline bf16 3-term kernel: 44011 ns, 1.1e-3).
"""

import numpy as np

import concourse.bass as bass
import concourse.mybir as mybir
from concourse.ap import AP
from concourse.tile import TileContext
from concourse.bass_utils import run_bass_kernel_spmd

# problem dims (hardcoded per the task contract)
B, C, OUT, H, W = 8, 32, 32, 64, 64
KBITS = 8
Q = 12.5
WVEC = np.array([-128, 1, 2, 4, 8, 16, 32, 64], np.float32)
SCALE = float(1.0 / (127.0 * Q))

PW = 66            # padded width  (1 + 64 + 1)
PH = 66            # padded height
PSZ = PH * PW      # 4356
NG = 4             # stage-1 passes: 4 x (4 segments x 2 channels)
KC = 120           # contraction rows per pass (last segment unpadded)
RPC = 12           # contraction rows per channel

BLK_ROWS = [3, 5, 6, 8, 8, 8, 8, 8, 8, 2]         # image rows per block
NPB = len(BLK_ROWS)
BLK_R0 = np.cumsum([0] + BLK_ROWS).tolist()        # first image row
MAXR = max(BLK_ROWS)
NDEN = MAXR * W    # max dense moving dim: 512 = one PSUM bank exactly
PADW = 64 * PW + 134  # host pad buffer width

NWARM = 30          # PE p-state warmup matmuls (128 cols each)
WCW = 2 * NG * 128 + 64  # weight cols: 8 stage-1 lhsT blocks + gmat
BOOTW = WCW + NG * BLK_ROWS[0] * PW  # boot: weights + pixel block 0
XREPW = NG * (H - BLK_ROWS[0]) * PW  # xrep dram: blocks 1..NPB-1

F16 = mybir.dt.float16
F32 = mybir.dt.float32

_cache = {}


def _f16(a):
    return np.asarray(a, np.float32).astype(np.float16)


def _row_of(c, j):
    """Contraction row (pass g, partition w) for channel c, intra row j."""
    g, r = divmod(c, 8)
    s, t = divmod(r, 2)
    return g, s * 32 + t * RPC + j


def _prep_weights(weight, bias):
    """Weight cols [128, WCW] fp16: 8 stage-1 lhsT blocks (g,m) + gmat."""
    qw = _f16(Q * weight.astype(np.float32)).reshape(KBITS, C, 2, 16, 9)
    qb = _f16(Q * bias.astype(np.float32)).reshape(KBITS, C, 2, 16)

    Wt = np.zeros((NG, KC, 2, 128), np.float16)
    for c in range(C):
        for j in range(9):
            g, w = _row_of(c, j)
            # qw[k,c,m,ol] -> [m, (k*16+ol)]
            Wt[g, w] = qw[:, c, :, :, j].transpose(1, 0, 2).reshape(2, 128)
        g, w = _row_of(c, 9)
        Wt[g, w] = qb[:, c].transpose(1, 0, 2).reshape(2, 128)
        g, w = _row_of(c, 10)
        Wt[g, w] = np.float16(3072.0)
        g, w = _row_of(c, 11)
        Wt[g, w] = np.float16(-3072.0)

    wc = np.zeros((128, WCW), np.float16)
    for g in range(NG):
        for m in range(2):
            wc[:KC, (g * 2 + m) * 128:(g * 2 + m + 1) * 128] = Wt[g, :, m, :]

    j = np.arange(128)
    k_of, ol_of = j // 16, j % 16
    gmat = np.zeros((128, 64), np.float32)
    gmat[j, ol_of] = WVEC[k_of] * SCALE           # A0 -> out rows 0..15
    gmat[j, 32 + 16 + ol_of] = WVEC[k_of] * SCALE  # A1 -> out rows 16..31
    wc[:, 2 * NG * 128:] = _f16(gmat)
    return wc


def _build_xrep(x):
    """Host REP: [B, KC, sum_blocks NG*window] fp16, block-sliced."""
    xh = _f16(x)
    xpad = np.zeros((B, C, PADW), np.float16)
    xpad[:, :, :PSZ].reshape(B, C, PH, PW)[:, :, 1:H + 1, 1:W + 1] = xh

    rep = np.zeros((B, NG, KC, H * PW), np.float16)
    for j in range(9):
        off = (j // 3) * PW + (j % 3)
        for c in range(C):
            g, w = _row_of(c, j)
            rep[:, g, w, :] = xpad[:, c, off:off + H * PW]
    for c in range(C):
        g, w = _row_of(c, 9)
        rep[:, g, w, :] = np.float16(1.0)
        for j in (10, 11):
            g, w = _row_of(c, j)
            rep[:, g, w, :] = np.float16(4096.0)
    # block-sliced, per block: [KC, NG * nr * PW]
    parts = []
    for pb in range(NPB):
        r0, nr = BLK_R0[pb], BLK_ROWS[pb]
        sl = rep[:, :, :, r0 * PW:(r0 + nr) * PW]           # [B, NG, KC, w]
        parts.append(sl.transpose(0, 2, 1, 3).reshape(B, KC, NG * nr * PW))
    return parts


def _split_multiwaits(nc):
    """This container's walrus allows one sync-wait per instruction; move
    extras onto preceding same-engine NoOps."""
    for bb in nc.main_func.blocks:
        insts = bb.instructions
        i = 0
        while i < len(insts):
            ins = insts[i]
            si = getattr(ins, "sync_info", None)
            if si is not None and si.on_wait is not None and len(si.on_wait) > 1:
                waits = list(si.on_wait)
                nops = []
                for j, w in enumerate(waits[:-1]):
                    nop = mybir.InstNoOp(name=f"{ins.name}-wsplit{j}", ins=[], outs=[])
                    nop.engine = ins.engine
                    nop.sync_info = mybir.SyncInfo(on_wait=[w], on_update=[])
                    nops.append(nop)
                si.on_wait = [waits[-1]]
                ins.sync_info = si
                for j, nop in enumerate(nops):
                    insts.insert(i + j, nop)
                i += len(nops)
            i += 1


def _build_nc():
    nc = bass.Bass()
    boot_d = nc.dram_tensor("boot", [128, BOOTW], F16, kind="ExternalInput")
    xrep_d = nc.dram_tensor("xrep", [KC, XREPW], F16, kind="ExternalInput")
    out_d = nc.dram_tensor("out", [OUT, H * W], F16, kind="ExternalOutput")

    with TileContext(nc) as tc:
        with (
            tc.tile_pool(name="const", bufs=1) as cpool,
            tc.tile_pool(name="blk", bufs=1) as bpool,
            tc.tile_pool(name="work", bufs=4) as wpool,
            tc.tile_pool(name="outp", bufs=2) as opool,
            tc.tile_pool(name="psP", bufs=4, space="PSUM") as psP,
            tc.tile_pool(name="psR", bufs=3, space="PSUM") as psR,
        ):
            # warmup: PE p-state ramp on a zero tile (memset on the
            # otherwise-idle Pool engine for the shortest dep latency)
            warm = cpool.tile([128, 128], F16, tag="warm")
            nc.gpsimd.memset(warm[:, :], 0.0)
            warm_ps = psP.tile([128, NDEN], F32, tag="P", name="warmps")
            for wi in range(NWARM):
                nc.tensor.matmul(warm_ps[:, :128], warm[:, :], warm[:, :],
                                 start=True, stop=True)

            boot = cpool.tile([128, BOOTW], F16, tag="boot")
            nc.sync.dma_start(out=boot[:, :], in_=boot_d[:, :])
            wconst = boot[:, 0:WCW]

            blk = [None] * NPB
            off_x = 0
            for pb in range(1, NPB):
                w = NG * BLK_ROWS[pb] * PW
                blk[pb] = bpool.tile([KC, w], F16, tag=f"blk{pb}",
                                     name=f"blk{pb}")
                src = AP(tensor=xrep_d, offset=off_x, ap=[[XREPW, KC], [1, w]])
                dst = AP(tensor=blk[pb].tensor, offset=blk[pb].offset,
                         ap=[[w, KC], [1, w]])
                nc.sync.dma_start(out=dst, in_=src)
                off_x += w

            def stage1(pb):
                nr = BLK_ROWS[pb]
                n = nr * W
                gw = nr * PW
                P = [psP.tile([128, NDEN], F32, tag="P", name=f"P{pb}_{m}")
                     for m in range(2)]
                # last two blocks run m-outer so P0's chain (and its copy)
                # completes 4 matmuls earlier, hiding copy latency in the
                # tail; earlier blocks stay m-inner (m-outer there opens a
                # PE gap at the block-1 data-arrival floor)
                gm_order = ([(g, m) for g in range(NG) for m in range(2)]
                            if pb < NPB - 2 else
                            [(g, m) for m in range(2) for g in range(NG)])
                for g, m in gm_order:
                    if pb == 0:
                        t, off, pitch = boot, boot.offset + WCW + g * gw, BOOTW
                    else:
                        t = blk[pb]
                        off, pitch = t.offset + g * gw, NG * gw
                    rhs = AP(tensor=t.tensor, offset=off,
                             ap=[[pitch, KC], [PW, nr], [1, W]])
                    lt = boot[0:KC, (g * 2 + m) * 128:(g * 2 + m + 1) * 128]
                    nc.tensor.matmul(P[m][:, :n], lt, rhs,
                                     start=(g == 0), stop=(g == NG - 1))
                A = [wpool.tile([128, NDEN], F16, tag="A", name=f"A{pb}_{m}")
                     for m in range(2)]
                # last (tiny) block: both copies on DVE — the scalar engine
                # is still busy with the previous block's copy
                nc.vector.tensor_copy(A[0][:, :n], P[0][:, :n])
                if pb == NPB - 1:
                    nc.vector.tensor_copy(A[1][:, :n], P[1][:, :n])
                else:
                    nc.scalar.copy(A[1][:, :n], P[1][:, :n])
                return A

            osb = None
            osb_off = 0
            osb_pb0 = 0
            # output-DMA pairing: if NPB is odd, block 0 goes solo (its DMA
            # fires early and costs nothing); later blocks pair up so the
            # tail ends with exactly one DMA
            pair_start = [True] + [(pb % 2) == (NPB % 2) for pb in range(1, NPB)]

            def stage2(pb, A):
                nonlocal osb, osb_off, osb_pb0
                nr = BLK_ROWS[pb]
                n = nr * W
                R = psR.tile([32, NDEN], F32, tag="R", name=f"R{pb}")
                gm0 = wconst[:, 2 * NG * 128:2 * NG * 128 + 32]
                gm1 = wconst[:, 2 * NG * 128 + 32:2 * NG * 128 + 64]
                nc.tensor.matmul(R[:, :n], gm0, A[0][:, :n],
                                 start=True, stop=False)
                nc.tensor.matmul(R[:, :n], gm1, A[1][:, :n],
                                 start=False, stop=True)
                if pair_start[pb]:
                    osb = opool.tile([32, 2 * NDEN], F16, tag="osb",
                                     name=f"osb{pb}")
                    osb_off = 0
                    osb_pb0 = pb
                out_ap = AP(tensor=osb.tensor, offset=osb.offset + osb_off,
                            ap=[[2 * NDEN, 32], [1, n]])
                # alternate osb copies between engines; swap for the final
                # pair so the big half lands on the faster scalar engine
                on_dve = (pb % 2 == 0) if pb < NPB - 2 else (pb % 2 == 1)
                if on_dve:
                    nc.vector.tensor_copy(out_ap, R[:, :n])
                else:
                    nc.scalar.copy(out_ap, R[:, :n])
                osb_off += n
                if pb == NPB - 1 or pair_start[pb + 1]:
                    pb0 = osb_pb0
                    nrows = sum(BLK_ROWS[pb0:pb + 1])
                    dst = AP(tensor=out_d, offset=BLK_R0[pb0] * W,
                             ap=[[H * W, OUT], [1, nrows * W]])
                    src = AP(tensor=osb.tensor, offset=osb.offset,
                             ap=[[2 * NDEN, 32], [1, nrows * W]])
                    nc.sync.dma_start(out=dst, in_=src)

            prevA = None
            for pb in range(NPB):
                A = stage1(pb)
                if prevA is not None:
                    stage2(pb - 1, prevA)
                prevA = A
            stage2(NPB - 1, prevA)

    _split_multiwaits(nc)
    return nc


def kernel(x, weight, bias):
    x = np.asarray(x, np.float32)
    weight = np.asarray(weight, np.float32)
    bias = np.asarray(bias, np.float32)

    parts = _build_xrep(x)
    wc = _prep_weights(weight, bias)

    if "nc" not in _cache:
        _cache["nc"] = _build_nc()
    nc = _cache["nc"]

    in_maps = []
    for b in range(B):
        boot = np.zeros((128, BOOTW), np.float16)
        boot[:, :WCW] = wc
        boot[:KC, WCW:] = parts[0][b]
        xr = np.concatenate([p[b] for p in parts[1:]], axis=1)
        in_maps.append({"boot": boot, "xrep": xr})
    res = run_bass_kernel_spmd(nc, in_maps, core_ids=list(range(B)))
    out = np.stack([r["out"] for r in res.results])
    return out.reshape(B, OUT, H, W).astype(np.float32)
